# revision 1
# baseline (speedup 1.0000x reference)
"""Trainium2 Bass kernel for causal multi-head attention with QKV/O projections.

Problem: x [1, 2048, 1024] f32, W_qkv [1024, 3072] (q|k|v blocks), W_o
[1024, 1024], H=16 heads, head_dim=64, dense causal attention,
y = softmax(q k^T / 8, causal) v, out = y @ W_o.

Sharding: head-parallel over 8 NeuronCores (2 heads per core). Each core
computes q/k/v projections for its 2 heads, causal attention, and a partial
O-projection (its 128 attention-output columns against its 128 rows of W_o).
The host sums the 8 partial outputs.

On-core dataflow (bf16 into the PE, f32 accumulation in PSUM):
  - xT [D, T] arrives pre-transposed from the host, so projections need no
    on-chip transposes:
       qT/kT [128, T] = W.T @ xT       (2 heads stacked on partitions)
       v     [T, 128] = x @ Wv         (lhsT = xT tiles)
    v is stored with a constant-1 column appended per head ([v_h | 1]), so
    the attention-V matmul also accumulates the softmax denominator.
  - attention is computed transposed: S_T [tk, tq] = kT-tile.T @ qT-tile,
    P_T = exp(S_T/8) in one ACT op per (tk, tq-block) position covering both
    heads (no max subtraction; |S| <= ~4 for this data), causal mask applied
    on diagonal 128x128 blocks, fully-masked blocks skipped.
  - numer_T/den: [65, tq] = [v_h | 1].T @ P_T per head. The denominator row
    is broadcast across 64 partitions with a K=1 fp32 matmul against a
    column of ones, reciprocal'd on DVE, and one elementwise multiply
    produces the normalized attention output (no cross-partition reductions).
  - the normalized numer_T is exactly the O-projection lhsT: y_partial
    [T, D] = att.T.T @ wo_rows, evacuated bf16 and summed on the host.

Work is emitted in 4 rounds (projection column-block n -> attention
tq-block j=n -> O-projection rows), so ACT's exp stream overlaps the PE's
projection matmuls of the next round.
"""

from contextlib import ExitStack

import numpy as np
import ml_dtypes

import concourse.bacc as bacc
import concourse.mybir as mybir
import concourse.tile as tile

BF16 = ml_dtypes.bfloat16
T = 2048
D = 1024
HD = 64
N_CORES = 8
KD = D // 128          # 8 contraction chunks for projections
NT128 = T // 128       # 16
NT512 = T // 512       # 4
VS = 130               # v_sb per-tile stride: [v_h0(64) | 1 | v_h1(64) | 1]
SCALE = 1.0 / 8.0      # 1/sqrt(64)

F32 = mybir.dt.float32
BF = mybir.dt.bfloat16


def _kernel(tc, y, xT, wq, wk, wv, wo, mask, dbg=None):
    nc = tc.nc
    Exp = mybir.ActivationFunctionType.Exp

    with ExitStack() as ctx:
        persist = ctx.enter_context(tc.tile_pool(name="persist", bufs=1))
        ps_mm = ctx.enter_context(tc.tile_pool(name="ps_mm", bufs=2, space="PSUM"))
        ps_s = ctx.enter_context(tc.tile_pool(name="ps_s", bufs=2, space="PSUM"))
        ps_av = ctx.enter_context(tc.tile_pool(name="ps_av", bufs=1, space="PSUM"))
        pool_p = ctx.enter_context(tc.tile_pool(name="pool_p", bufs=5))
        pool_r = ctx.enter_context(tc.tile_pool(name="pool_r", bufs=2))
        pool_y = ctx.enter_context(tc.tile_pool(name="pool_y", bufs=4))

        # ---- small inputs first so the PE can start as soon as xT trickles in
        wq_sb = persist.tile([128, D], BF, tag="wq")
        nc.sync.dma_start(wq_sb[:], wq[:])
        wk_sb = persist.tile([128, D], BF, tag="wk")
        nc.sync.dma_start(wk_sb[:], wk[:])
        wv_sb = persist.tile([128, D], BF, tag="wv")
        nc.gpsimd.dma_start(wv_sb[:], wv[:])
        wo_sb = persist.tile([128, D], BF, tag="wo")
        nc.gpsimd.dma_start(wo_sb[:], wo[:])
        mask_sb = persist.tile([128, 128], BF, tag="mask")
        nc.gpsimd.dma_start(mask_sb[:], mask[:])

        xT_sb = persist.tile([128, KD * T], BF, tag="xT")  # d-chunk d at cols [d*T,(d+1)*T)
        for d in range(KD):
            eng = nc.sync if d % 2 == 0 else nc.gpsimd
            eng.dma_start(xT_sb[:, d * T:(d + 1) * T], xT[d * 128:(d + 1) * 128, :])

        qT_sb = persist.tile([128, T], BF, tag="qT")   # partitions 0-63 head0, 64-127 head1
        kT_sb = persist.tile([128, T], BF, tag="kT")
        v_sb = persist.tile([128, NT128 * VS], BF, tag="v")
        nc.vector.memset(v_sb[:], 1.0)                 # pre-set the ones columns
        ones32 = persist.tile([65, HD], F32, tag="ones32")
        nc.vector.memset(ones32[:], 1.0)
        att_sb = persist.tile([128, T], BF, tag="att")  # normalized numer_T

        for rnd in range(NT512):
            # ---- qT / kT projection column-block rnd ----
            for w_sb, dst in ((wq_sb, qT_sb), (wk_sb, kT_sb)):
                ps = ps_mm.tile([128, 512], F32, tag="mm")
                for d in range(KD):
                    nc.tensor.matmul(
                        ps[:],
                        lhsT=w_sb[:, d * 128:(d + 1) * 128],
                        rhs=xT_sb[:, d * T + rnd * 512: d * T + (rnd + 1) * 512],
                        start=(d == 0), stop=(d == KD - 1),
                    )
                nc.vector.tensor_copy(dst[:, rnd * 512:(rnd + 1) * 512], ps[:])

            # ---- v projection tiles of this round: v[t] [128,128] = x @ Wv ----
            for t in range(4 * rnd, 4 * rnd + 4):
                ps = ps_mm.tile([128, 512], F32, tag="mm")
                for d in range(KD):
                    nc.tensor.matmul(
                        ps[:, 0:128],
                        lhsT=xT_sb[:, d * T + t * 128: d * T + (t + 1) * 128],
                        rhs=wv_sb[:, d * 128:(d + 1) * 128],
                        start=(d == 0), stop=(d == KD - 1),
                    )
                # one strided cast fills v_h0 -> cols [VS*t, +64) and
                # v_h1 -> cols [VS*t+65, +64), leaving the ones columns intact
                dst = v_sb[:, VS * t: VS * t + VS].rearrange("p (a b) -> p a b", b=65)[:, :, 0:64]
                src = ps[:, 0:128].rearrange("p (a b) -> p a b", b=64)
                nc.vector.tensor_copy(dst, src)

            # ---- attention for tq block j = rnd ----
            j = rnd
            avden = ps_av.tile([128, 1024], F32, tag="avden")  # bank per head: [65, 512] used
            n_i = 4 * j + 4
            for i in range(n_i):
                m = i - 4 * j          # >= 0 on diagonal blocks
                off = 128 * m if m > 0 else 0
                ncol = 512 - off
                first, last = (i == 0), (i == n_i - 1)
                # h0 always full width; h1 column-trimmed so one ACT op covers
                # [0, 512+ncol) contiguously with no uninitialized gap.
                s_pair = ps_s.tile([128, 1024], F32, tag="s")
                nc.tensor.matmul(
                    s_pair[:, 0:512],
                    lhsT=kT_sb[0:64, i * 128:(i + 1) * 128],
                    rhs=qT_sb[0:64, j * 512:(j + 1) * 512],
                    start=True, stop=True, tile_position=(0, 0),
                )
                nc.tensor.matmul(
                    s_pair[:, 512:512 + ncol],
                    lhsT=kT_sb[64:128, i * 128:(i + 1) * 128],
                    rhs=qT_sb[64:128, j * 512 + off: (j + 1) * 512],
                    start=True, stop=True, tile_position=(64, 0),
                )
                p_sb = pool_p.tile([128, 1024], BF, tag="p")
                nc.scalar.activation(
                    p_sb[:, 0:512 + ncol], s_pair[:, 0:512 + ncol], Exp, scale=SCALE,
                )
                if m >= 0:  # causal mask on the 128x128 diagonal sub-block
                    nc.vector.tensor_mul(
                        p_sb[:, 128 * m:128 * m + 128],
                        p_sb[:, 128 * m:128 * m + 128], mask_sb[:],
                    )
                    nc.vector.tensor_mul(
                        p_sb[:, 512:640], p_sb[:, 512:640], mask_sb[:],
                    )
                nc.tensor.matmul(
                    avden[0:65, off:512],
                    lhsT=v_sb[:, VS * i: VS * i + 65],
                    rhs=p_sb[:, off:512],
                    start=first, stop=last,
                )
                nc.tensor.matmul(
                    avden[0:65, 512 + off:1024],
                    lhsT=v_sb[:, VS * i + 65: VS * i + 130],
                    rhs=p_sb[:, 512:512 + ncol],
                    start=first, stop=last,
                )

            # ---- normalize: row 64 of each head's bank is the denominator ----
            for h in range(2):
                hc = h * 512
                denrow = pool_r.tile([65, 512], F32, tag="denrow")
                nc.vector.tensor_copy(denrow[64:65, :], avden[64:65, hc:hc + 512])
                bc_ps = ps_mm.tile([128, 512], F32, tag="mm")
                nc.tensor.matmul(
                    bc_ps[0:64, :], lhsT=ones32[64:65, :], rhs=denrow[64:65, :],
                    start=True, stop=True,
                )
                recip = pool_r.tile([64, 512], F32, tag="recip")
                nc.vector.reciprocal_approx_fast(recip[:], bc_ps[0:64, :])
                nc.vector.tensor_mul(
                    att_sb[h * 64:(h + 1) * 64, j * 512:(j + 1) * 512],
                    avden[0:64, hc:hc + 512], recip[:],
                )

            # ---- O-projection for the 4 T-chunks of this block ----
            for t in range(4 * j, 4 * j + 4):
                for nh in range(2):
                    ps = ps_mm.tile([128, 512], F32, tag="mm")
                    nc.tensor.matmul(
                        ps[:],
                        lhsT=att_sb[:, t * 128:(t + 1) * 128],
                        rhs=wo_sb[:, nh * 512:(nh + 1) * 512],
                        start=True, stop=True,
                    )
                    y_sb = pool_y.tile([128, 512], BF, tag="y")
                    if nh == 0:
                        nc.vector.tensor_copy(y_sb[:], ps[:])
                    else:
                        nc.scalar.copy(y_sb[:], ps[:])
                    eng = nc.sync if t % 2 == 0 else nc.gpsimd
                    eng.dma_start(
                        y[t * 128:(t + 1) * 128, nh * 512:(nh + 1) * 512], y_sb[:]
                    )

        if dbg is not None:
            for name, sb in (("qT", qT_sb), ("kT", kT_sb), ("att", att_sb)):
                nc.sync.dma_start(dbg[name][:], sb[:])


def _build_program(debug_dumps=False):
    nc = bacc.Bacc("TRN2", debug=False, num_devices=N_CORES)
    xT = nc.dram_tensor("xT", [D, T], BF, kind="ExternalInput").ap()
    wq = nc.dram_tensor("wq", [128, D], BF, kind="ExternalInput").ap()
    wk = nc.dram_tensor("wk", [128, D], BF, kind="ExternalInput").ap()
    wv = nc.dram_tensor("wv", [128, D], BF, kind="ExternalInput").ap()
    wo = nc.dram_tensor("wo", [128, D], BF, kind="ExternalInput").ap()
    mask = nc.dram_tensor("mask", [128, 128], BF, kind="ExternalInput").ap()
    y = nc.dram_tensor("y", [T, D], BF, kind="ExternalOutput").ap()
    dbg = None
    if debug_dumps:
        dbg = {
            name: nc.dram_tensor(f"dbg_{name}", [128, T], BF, kind="ExternalOutput").ap()
            for name in ("qT", "kT", "att")
        }

    with tile.TileContext(nc) as tc:
        _kernel(tc, y, xT, wq, wk, wv, wo, mask, dbg=dbg)
    nc.compile()
    return nc


_NC = None


def _get_program():
    global _NC
    if _NC is None:
        _NC = _build_program()
    return _NC


def _rearrange_w(w_cols):
    """[1024, 128] f32 slice of W_qkv -> [128, 1024] bf16 with d-chunk d at
    cols [d*128, (d+1)*128): out[p, d*128 + m] = w_cols[d*128 + p, m]."""
    return np.ascontiguousarray(
        w_cols.reshape(KD, 128, 128).transpose(1, 0, 2).reshape(128, KD * 128)
    ).astype(BF16)


def make_in_maps(x, W_qkv, W_o):
    x2 = np.asarray(x, dtype=np.float32).reshape(T, D)
    W_qkv = np.asarray(W_qkv, dtype=np.float32)
    W_o = np.asarray(W_o, dtype=np.float32)

    xT_bf = np.ascontiguousarray(x2.T).astype(BF16)
    mask = np.triu(np.ones((128, 128), dtype=np.float32)).astype(BF16)

    in_maps = []
    for c in range(N_CORES):
        cs = slice(2 * c * HD, 2 * c * HD + 128)
        in_maps.append({
            "xT": xT_bf,
            "wq": _rearrange_w(W_qkv[:, 0 * D:1 * D][:, cs]),
            "wk": _rearrange_w(W_qkv[:, 1 * D:2 * D][:, cs]),
            "wv": _rearrange_w(W_qkv[:, 2 * D:3 * D][:, cs]),
            "wo": np.ascontiguousarray(W_o[c * 128:(c + 1) * 128, :]).astype(BF16),
            "mask": mask,
        })
    return in_maps


def combine_outputs(results):
    y_full = np.zeros((T, D), dtype=np.float32)
    for c in range(N_CORES):
        y_full += results[c]["y"].astype(np.float32)
    return y_full.reshape(1, T, D)


def kernel(x, W_qkv, W_o):
    from concourse.bass_utils import run_bass_kernel_spmd

    nc = _get_program()
    in_maps = make_in_maps(x, W_qkv, W_o)
    res = run_bass_kernel_spmd(nc, in_maps, core_ids=list(range(N_CORES)))
    return combine_outputs(res.results)



# revision 7
# speedup vs baseline: 1.0838x; 1.0838x over previous
"""Trainium2 Bass kernel for causal multi-head attention with QKV/O projections.

Problem: x [1, 2048, 1024] f32, W_qkv [1024, 3072] (q|k|v blocks), W_o
[1024, 1024], H=16 heads, head_dim=64, dense causal attention,
y = softmax(q k^T / 8, causal) v, out = y @ W_o.

Sharding: head-parallel over 8 NeuronCores (2 heads per core). Each core
computes q/k/v projections for its 2 heads, causal attention, and a partial
O-projection (its 128 attention-output columns against its 128 rows of W_o).
The host sums the 8 partial outputs.

On-core dataflow (bf16 into the PE, f32 accumulation in PSUM):
  - xT [D, T] arrives pre-transposed from the host, so projections need no
    on-chip transposes:
       qT/kT [128, T] = W.T @ xT       (2 heads stacked on partitions)
       v     [T, 128] = x @ Wv         (lhsT = xT tiles)
    v is stored with a constant-1 column appended per head ([v_h | 1]), so
    the attention-V matmul also accumulates the softmax denominator.
  - attention is computed transposed: S_T [tk, tq] = kT-tile.T @ qT-tile,
    P_T = exp(S_T/8) in one ACT op per (tk, tq-block) position covering both
    heads (no max subtraction; |S| <= ~4 for this data), causal mask applied
    on diagonal 128x128 blocks, fully-masked blocks skipped.
  - numer_T/den: [65, tq] = [v_h | 1].T @ P_T per head. The denominator row
    is broadcast across 64 partitions with a K=1 fp32 matmul against a
    column of ones, reciprocal'd on DVE, and one elementwise multiply
    produces the normalized attention output (no cross-partition reductions).
  - the normalized numer_T is exactly the O-projection lhsT: y_partial
    [T, D] = att.T.T @ wo_rows, evacuated bf16 and summed on the host.

Work is emitted in 4 rounds (projection column-block n -> attention
tq-block j=n -> O-projection rows), so ACT's exp stream overlaps the PE's
projection matmuls of the next round.
"""

from contextlib import ExitStack

import numpy as np
import ml_dtypes

import concourse.bacc as bacc
import concourse.mybir as mybir
import concourse.tile as tile

BF16 = ml_dtypes.bfloat16
T = 2048
D = 1024
HD = 64
N_CORES = 8
KD = D // 128          # 8 contraction chunks for projections
NT128 = T // 128       # 16
NT512 = T // 512       # 4
VS = 130               # v_sb per-tile stride: [v_h0(64) | 1 | v_h1(64) | 1]
SCALE = 1.0 / 8.0      # 1/sqrt(64)

F32 = mybir.dt.float32
BF = mybir.dt.bfloat16


def _kernel(tc, y, xT, wq, wk, wv, wo, mask, dbg=None):
    nc = tc.nc
    Exp = mybir.ActivationFunctionType.Exp

    with ExitStack() as ctx:
        persist = ctx.enter_context(tc.tile_pool(name="persist", bufs=1))
        ps_mm = ctx.enter_context(tc.tile_pool(name="ps_mm", bufs=2, space="PSUM"))
        ps_s = ctx.enter_context(tc.tile_pool(name="ps_s", bufs=2, space="PSUM"))
        ps_av = ctx.enter_context(tc.tile_pool(name="ps_av", bufs=1, space="PSUM"))
        pool_p = ctx.enter_context(tc.tile_pool(name="pool_p", bufs=5))
        pool_r = ctx.enter_context(tc.tile_pool(name="pool_r", bufs=2))
        pool_y = ctx.enter_context(tc.tile_pool(name="pool_y", bufs=4))

        # trigger the ACT exp table-set DMA (~2.7us) during the input-DMA head
        warm_in = persist.tile([1, 8], F32, tag="warm_in")
        nc.vector.memset(warm_in[:], 0.0)
        warm_out = persist.tile([1, 8], F32, tag="warm_out")
        nc.scalar.activation(warm_out[:], warm_in[:], Exp)

        # ---- all input DMAs on the sync HWDGE ring, in the order the PE
        # consumes them: round-0 operands first so matmuls start ~1.5us in.
        wq_sb = persist.tile([128, D], BF, tag="wq")
        nc.sync.dma_start(wq_sb[:], wq[:])
        wk_sb = persist.tile([128, D], BF, tag="wk")
        nc.sync.dma_start(wk_sb[:], wk[:])

        xT_sb = persist.tile([128, KD * T], BF, tag="xT")  # d-chunk d at cols [d*T,(d+1)*T)

        def dma_xT(rnd):
            for d in range(KD):
                nc.sync.dma_start(
                    xT_sb[:, d * T + rnd * 512: d * T + (rnd + 1) * 512],
                    xT[d * 128:(d + 1) * 128, rnd * 512:(rnd + 1) * 512],
                )

        dma_xT(0)
        wv_sb = persist.tile([128, D], BF, tag="wv")
        nc.sync.dma_start(wv_sb[:], wv[:])
        mask_sb = persist.tile([128, 128], BF, tag="mask")
        nc.sync.dma_start(mask_sb[:], mask[:])
        dma_xT(1)
        wo_sb = persist.tile([128, D], BF, tag="wo")
        nc.sync.dma_start(wo_sb[:], wo[:])
        dma_xT(2)
        dma_xT(3)

        qT_sb = persist.tile([128, T], BF, tag="qT")   # partitions 0-63 head0, 64-127 head1
        kT_sb = persist.tile([128, T], BF, tag="kT")
        v_sb = persist.tile([128, NT128 * VS], BF, tag="v")
        nc.vector.memset(v_sb[:], 1.0)                 # pre-set the ones columns
        ones32 = persist.tile([65, HD], F32, tag="ones32")
        nc.vector.memset(ones32[:], 1.0)
        att_sb = persist.tile([128, T], BF, tag="att")  # normalized numer_T

        for rnd in range(NT512):
            # ---- qT / kT projection column-block rnd, q/k interleaved per
            # d-chunk so the first matmuls pace with the xT DMA arrivals ----
            ps_q = ps_mm.tile([128, 512], F32, tag="mm")
            ps_k = ps_mm.tile([128, 512], F32, tag="mm")
            for d in range(KD):
                for w_sb, ps in ((wq_sb, ps_q), (wk_sb, ps_k)):
                    nc.tensor.matmul(
                        ps[:],
                        lhsT=w_sb[:, d * 128:(d + 1) * 128],
                        rhs=xT_sb[:, d * T + rnd * 512: d * T + (rnd + 1) * 512],
                        start=(d == 0), stop=(d == KD - 1),
                    )
            nc.vector.tensor_copy(qT_sb[:, rnd * 512:(rnd + 1) * 512], ps_q[:])
            nc.vector.tensor_copy(kT_sb[:, rnd * 512:(rnd + 1) * 512], ps_k[:])

            # ---- v projection tiles of this round: v[t] [128,128] = x @ Wv ----
            for t in range(4 * rnd, 4 * rnd + 4):
                ps = ps_mm.tile([128, 512], F32, tag="mm")
                for d in range(KD):
                    nc.tensor.matmul(
                        ps[:, 0:128],
                        lhsT=xT_sb[:, d * T + t * 128: d * T + (t + 1) * 128],
                        rhs=wv_sb[:, d * 128:(d + 1) * 128],
                        start=(d == 0), stop=(d == KD - 1),
                    )
                # one strided cast fills v_h0 -> cols [VS*t, +64) and
                # v_h1 -> cols [VS*t+65, +64), leaving the ones columns intact
                dst = v_sb[:, VS * t: VS * t + VS].rearrange("p (a b) -> p a b", b=65)[:, :, 0:64]
                src = ps[:, 0:128].rearrange("p (a b) -> p a b", b=64)
                nc.vector.tensor_copy(dst, src)

            # ---- attention for tq block j = rnd ----
            j = rnd
            avden = ps_av.tile([128, 1024], F32, tag="avden")  # bank per head: [65, 512] used
            n_i = 4 * j + 4
            for i in range(n_i):
                m = i - 4 * j          # >= 0 on diagonal blocks
                off = 128 * m if m > 0 else 0
                ncol = 512 - off
                first, last = (i == 0), (i == n_i - 1)
                # both heads causally column-trimmed; one ACT op covers
                # [off, 512+ncol) contiguously with no uninitialized gap.
                s_pair = ps_s.tile([128, 1024], F32, tag="s")
                nc.tensor.matmul(
                    s_pair[:, off:512],
                    lhsT=kT_sb[0:64, i * 128:(i + 1) * 128],
                    rhs=qT_sb[0:64, j * 512 + off:(j + 1) * 512],
                    start=True, stop=True, tile_position=(0, 0),
                )
                nc.tensor.matmul(
                    s_pair[:, 512:512 + ncol],
                    lhsT=kT_sb[64:128, i * 128:(i + 1) * 128],
                    rhs=qT_sb[64:128, j * 512 + off: (j + 1) * 512],
                    start=True, stop=True, tile_position=(64, 0),
                )
                p_sb = pool_p.tile([128, 1024], BF, tag="p")
                nc.scalar.activation(
                    p_sb[:, off:512 + ncol], s_pair[:, off:512 + ncol], Exp, scale=SCALE,
                )
                if m >= 0:  # causal mask on the 128x128 diagonal sub-block
                    nc.vector.tensor_mul(
                        p_sb[:, off:off + 128],
                        p_sb[:, off:off + 128], mask_sb[:],
                    )
                    nc.vector.tensor_mul(
                        p_sb[:, 512:640], p_sb[:, 512:640], mask_sb[:],
                    )
                nc.tensor.matmul(
                    avden[0:65, off:512],
                    lhsT=v_sb[:, VS * i: VS * i + 65],
                    rhs=p_sb[:, off:512],
                    start=first, stop=last,
                )
                nc.tensor.matmul(
                    avden[0:65, 512 + off:1024],
                    lhsT=v_sb[:, VS * i + 65: VS * i + 130],
                    rhs=p_sb[:, 512:512 + ncol],
                    start=first, stop=last,
                )

            # ---- normalize: row 64 of each head's bank is the denominator ----
            for h in range(2):
                hc = h * 512
                denrow = pool_r.tile([65, 512], F32, tag="denrow")
                nc.vector.tensor_copy(denrow[64:65, :], avden[64:65, hc:hc + 512])
                bc_ps = ps_mm.tile([128, 512], F32, tag="mm")
                nc.tensor.matmul(
                    bc_ps[0:64, :], lhsT=ones32[64:65, :], rhs=denrow[64:65, :],
                    start=True, stop=True,
                )
                recip = pool_r.tile([64, 512], F32, tag="recip")
                nc.vector.reciprocal_approx_fast(recip[:], bc_ps[0:64, :])
                nc.vector.tensor_mul(
                    att_sb[h * 64:(h + 1) * 64, j * 512:(j + 1) * 512],
                    avden[0:64, hc:hc + 512], recip[:],
                )

            # ---- O-projection for the 4 T-chunks of this block ----
            for t in range(4 * j, 4 * j + 4):
                y_sb = pool_y.tile([128, 1024], BF, tag="y")
                for nh in range(2):
                    ps = ps_mm.tile([128, 512], F32, tag="mm")
                    nc.tensor.matmul(
                        ps[:],
                        lhsT=att_sb[:, t * 128:(t + 1) * 128],
                        rhs=wo_sb[:, nh * 512:(nh + 1) * 512],
                        start=True, stop=True,
                    )
                    nc.vector.tensor_copy(y_sb[:, nh * 512:(nh + 1) * 512], ps[:])
                nc.scalar.dma_start(y[t * 128:(t + 1) * 128, :], y_sb[:])

        if dbg is not None:
            for name, sb in (("qT", qT_sb), ("kT", kT_sb), ("att", att_sb)):
                nc.sync.dma_start(dbg[name][:], sb[:])


def _build_program(debug_dumps=False):
    nc = bacc.Bacc("TRN2", debug=False, num_devices=N_CORES)
    xT = nc.dram_tensor("xT", [D, T], BF, kind="ExternalInput").ap()
    wq = nc.dram_tensor("wq", [128, D], BF, kind="ExternalInput").ap()
    wk = nc.dram_tensor("wk", [128, D], BF, kind="ExternalInput").ap()
    wv = nc.dram_tensor("wv", [128, D], BF, kind="ExternalInput").ap()
    wo = nc.dram_tensor("wo", [128, D], BF, kind="ExternalInput").ap()
    mask = nc.dram_tensor("mask", [128, 128], BF, kind="ExternalInput").ap()
    y = nc.dram_tensor("y", [T, D], BF, kind="ExternalOutput").ap()
    dbg = None
    if debug_dumps:
        dbg = {
            name: nc.dram_tensor(f"dbg_{name}", [128, T], BF, kind="ExternalOutput").ap()
            for name in ("qT", "kT", "att")
        }

    with tile.TileContext(nc) as tc:
        _kernel(tc, y, xT, wq, wk, wv, wo, mask, dbg=dbg)
    nc.compile()
    return nc


_NC = None


def _get_program():
    global _NC
    if _NC is None:
        _NC = _build_program()
    return _NC


def _rearrange_w(w_cols):
    """[1024, 128] f32 slice of W_qkv -> [128, 1024] bf16 with d-chunk d at
    cols [d*128, (d+1)*128): out[p, d*128 + m] = w_cols[d*128 + p, m]."""
    return np.ascontiguousarray(
        w_cols.reshape(KD, 128, 128).transpose(1, 0, 2).reshape(128, KD * 128)
    ).astype(BF16)


def make_in_maps(x, W_qkv, W_o):
    x2 = np.asarray(x, dtype=np.float32).reshape(T, D)
    W_qkv = np.asarray(W_qkv, dtype=np.float32)
    W_o = np.asarray(W_o, dtype=np.float32)

    xT_bf = np.ascontiguousarray(x2.T).astype(BF16)
    mask = np.triu(np.ones((128, 128), dtype=np.float32)).astype(BF16)

    in_maps = []
    for c in range(N_CORES):
        cs = slice(2 * c * HD, 2 * c * HD + 128)
        in_maps.append({
            "xT": xT_bf,
            "wq": _rearrange_w(W_qkv[:, 0 * D:1 * D][:, cs]),
            "wk": _rearrange_w(W_qkv[:, 1 * D:2 * D][:, cs]),
            "wv": _rearrange_w(W_qkv[:, 2 * D:3 * D][:, cs]),
            "wo": np.ascontiguousarray(W_o[c * 128:(c + 1) * 128, :]).astype(BF16),
            "mask": mask,
        })
    return in_maps


def combine_outputs(results):
    y_full = np.zeros((T, D), dtype=np.float32)
    for c in range(N_CORES):
        y_full += results[c]["y"].astype(np.float32)
    return y_full.reshape(1, T, D)


def kernel(x, W_qkv, W_o):
    from concourse.bass_utils import run_bass_kernel_spmd

    nc = _get_program()
    in_maps = make_in_maps(x, W_qkv, W_o)
    res = run_bass_kernel_spmd(nc, in_maps, core_ids=list(range(N_CORES)))
    return combine_outputs(res.results)



# revision 8
# speedup vs baseline: 1.3803x; 1.2736x over previous
"""Trainium2 Bass kernel for causal multi-head attention with QKV/O projections.

Problem: x [1, 2048, 1024] f32, W_qkv [1024, 3072] (q|k|v blocks), W_o
[1024, 1024], H=16 heads, head_dim=64, dense causal attention,
y = softmax(q k^T / 8, causal) v, out = y @ W_o.

Sharding: head-parallel over 8 NeuronCores (2 heads per core). Each core
computes q/k/v projections for its 2 heads, causal attention, and a partial
O-projection (its 128 attention-output columns against its 128 rows of W_o).
The host sums the 8 partial outputs.

On-core dataflow (bf16 into the PE, f32 accumulation in PSUM):
  - xT [D, T] arrives pre-transposed from the host, so projections need no
    on-chip transposes:
       qT/kT [128, T] = W.T @ xT       (2 heads stacked on partitions)
       v     [T, 128] = x @ Wv         (lhsT = xT tiles)
    v is stored with a constant-1 column appended per head ([v_h | 1]), so
    the attention-V matmul also accumulates the softmax denominator.
  - attention is computed transposed: S_T [tk, tq] = kT-tile.T @ qT-tile,
    P_T = exp(S_T/8) in one ACT op per (tk, tq-block) position covering both
    heads (no max subtraction; |S| <= ~4 for this data), causal mask applied
    on diagonal 128x128 blocks, fully-masked blocks skipped.
  - numer_T/den: [65, tq] = [v_h | 1].T @ P_T per head. The denominator row
    is broadcast across 64 partitions with a K=1 fp32 matmul against a
    column of ones, reciprocal'd on DVE, and one elementwise multiply
    produces the normalized attention output (no cross-partition reductions).
  - the normalized numer_T is exactly the O-projection lhsT: y_partial
    [T, D] = att.T.T @ wo_rows, evacuated bf16 and summed on the host.

Work is emitted in 4 rounds (projection column-block n -> attention
tq-block j=n -> O-projection rows), so ACT's exp stream overlaps the PE's
projection matmuls of the next round.
"""

from contextlib import ExitStack

import numpy as np
import ml_dtypes

import concourse.bacc as bacc
import concourse.mybir as mybir
import concourse.tile as tile

BF16 = ml_dtypes.bfloat16
T = 2048
D = 1024
HD = 64
N_CORES = 8
KD = D // 128          # 8 contraction chunks for projections
NT128 = T // 128       # 16
NT512 = T // 512       # 4
VS = 130               # v_sb per-tile stride: [v_h0(64) | 1 | v_h1(64) | 1]
SCALE = 1.0 / 8.0      # 1/sqrt(64)

F32 = mybir.dt.float32
BF = mybir.dt.bfloat16


def _kernel(tc, y, xT, wq, wk, wv, wo, mask, dbg=None):
    nc = tc.nc
    Exp = mybir.ActivationFunctionType.Exp

    with ExitStack() as ctx:
        persist = ctx.enter_context(tc.tile_pool(name="persist", bufs=1))
        ps_mm = ctx.enter_context(tc.tile_pool(name="ps_mm", bufs=2, space="PSUM"))
        ps_s = ctx.enter_context(tc.tile_pool(name="ps_s", bufs=2, space="PSUM"))
        ps_av = ctx.enter_context(tc.tile_pool(name="ps_av", bufs=1, space="PSUM"))
        pool_p = ctx.enter_context(tc.tile_pool(name="pool_p", bufs=5))
        pool_r = ctx.enter_context(tc.tile_pool(name="pool_r", bufs=2))
        pool_y = ctx.enter_context(tc.tile_pool(name="pool_y", bufs=4))

        # trigger the ACT exp table-set DMA (~2.7us) during the input-DMA head
        warm_in = persist.tile([1, 8], F32, tag="warm_in")
        nc.vector.memset(warm_in[:], 0.0)
        warm_out = persist.tile([1, 8], F32, tag="warm_out")
        nc.scalar.activation(warm_out[:], warm_in[:], Exp)

        # ---- all input DMAs on the sync HWDGE ring, in the order the PE
        # consumes them: round-0 operands first so matmuls start ~1.5us in.
        wq_sb = persist.tile([128, D], BF, tag="wq")
        nc.sync.dma_start(wq_sb[:], wq[:])
        wk_sb = persist.tile([128, D], BF, tag="wk")
        nc.sync.dma_start(wk_sb[:], wk[:])

        xT_sb = persist.tile([128, KD * T], BF, tag="xT")  # d-chunk d at cols [d*T,(d+1)*T)

        def dma_xT(rnd):
            for d in range(KD):
                nc.sync.dma_start(
                    xT_sb[:, d * T + rnd * 512: d * T + (rnd + 1) * 512],
                    xT[d * 128:(d + 1) * 128, rnd * 512:(rnd + 1) * 512],
                )

        dma_xT(0)
        wv_sb = persist.tile([128, D], BF, tag="wv")
        nc.sync.dma_start(wv_sb[:], wv[:])
        mask_sb = persist.tile([128, 128], BF, tag="mask")
        nc.sync.dma_start(mask_sb[:], mask[:])
        dma_xT(1)
        wo_sb = persist.tile([128, D], BF, tag="wo")
        nc.sync.dma_start(wo_sb[:], wo[:])
        dma_xT(2)
        dma_xT(3)

        qT_sb = persist.tile([128, T], BF, tag="qT")   # partitions 0-63 head0, 64-127 head1
        kT_sb = persist.tile([128, T], BF, tag="kT")
        v_sb = persist.tile([128, NT128 * VS], BF, tag="v")
        nc.vector.memset(v_sb[:], 1.0)                 # pre-set the ones columns
        ones32 = persist.tile([65, HD], F32, tag="ones32")
        nc.vector.memset(ones32[:], 1.0)
        att_sb = persist.tile([128, T], BF, tag="att")  # normalized numer_T

        # ---- PE warm-up: dummy matmuls during the DMA head keep/get HAM hot
        # and cost nothing (PE is otherwise idle until the first xT piece).
        dummy_sb = persist.tile([128, 512], BF, tag="dummy")
        nc.gpsimd.memset(dummy_sb[:], 0.0)
        for _ in range(8):
            ps_warm = ps_s.tile([128, 1024], F32, tag="s")
            nc.tensor.matmul(
                ps_warm[:, 0:512], lhsT=dummy_sb[:, 0:128], rhs=dummy_sb[:],
                start=True, stop=True,
            )

        # ================= emission helpers =================
        def qk_proj_interleaved(rnd):
            """round-0 prologue: q/k interleaved per d-chunk to pace with DMA."""
            ps_q = ps_mm.tile([128, 512], F32, tag="mm")
            ps_k = ps_mm.tile([128, 512], F32, tag="mm")
            for d in range(KD):
                for w_sb, ps in ((wq_sb, ps_q), (wk_sb, ps_k)):
                    nc.tensor.matmul(
                        ps[:],
                        lhsT=w_sb[:, d * 128:(d + 1) * 128],
                        rhs=xT_sb[:, d * T + rnd * 512: d * T + (rnd + 1) * 512],
                        start=(d == 0), stop=(d == KD - 1),
                    )
            nc.vector.tensor_copy(qT_sb[:, rnd * 512:(rnd + 1) * 512], ps_q[:])
            nc.vector.tensor_copy(kT_sb[:, rnd * 512:(rnd + 1) * 512], ps_k[:])

        def qk_chain(rnd, w_sb, dst):
            """one projection chain (8 accumulating MMs) + evacuation."""
            ps = ps_mm.tile([128, 512], F32, tag="mm")
            for d in range(KD):
                nc.tensor.matmul(
                    ps[:],
                    lhsT=w_sb[:, d * 128:(d + 1) * 128],
                    rhs=xT_sb[:, d * T + rnd * 512: d * T + (rnd + 1) * 512],
                    start=(d == 0), stop=(d == KD - 1),
                )
            nc.vector.tensor_copy(dst[:, rnd * 512:(rnd + 1) * 512], ps[:])

        def v_tile(t):
            """one v tile [128,128] = x @ Wv + strided cast into v_sb."""
            ps = ps_mm.tile([128, 512], F32, tag="mm")
            for d in range(KD):
                nc.tensor.matmul(
                    ps[:, 0:128],
                    lhsT=xT_sb[:, d * T + t * 128: d * T + (t + 1) * 128],
                    rhs=wv_sb[:, d * 128:(d + 1) * 128],
                    start=(d == 0), stop=(d == KD - 1),
                )
            dst = v_sb[:, VS * t: VS * t + VS].rearrange("p (a b) -> p a b", b=65)[:, :, 0:64]
            src = ps[:, 0:128].rearrange("p (a b) -> p a b", b=64)
            nc.vector.tensor_copy(dst, src)

        def o_chunk(t):
            """O-projection for T-chunk t: 2 matmuls + casts + output DMA."""
            y_sb = pool_y.tile([128, 1024], BF, tag="y")
            for nh in range(2):
                ps = ps_mm.tile([128, 512], F32, tag="mm")
                nc.tensor.matmul(
                    ps[:],
                    lhsT=att_sb[:, t * 128:(t + 1) * 128],
                    rhs=wo_sb[:, nh * 512:(nh + 1) * 512],
                    start=True, stop=True,
                )
                nc.vector.tensor_copy(y_sb[:, nh * 512:(nh + 1) * 512], ps[:])
            nc.scalar.dma_start(y[t * 128:(t + 1) * 128, :], y_sb[:])

        def normalize(j, avden):
            """row 64 of each head's avden bank is the softmax denominator."""
            for h in range(2):
                hc = h * 512
                denrow = pool_r.tile([65, 512], F32, tag="denrow")
                nc.vector.tensor_copy(denrow[64:65, :], avden[64:65, hc:hc + 512])
                bc_ps = ps_mm.tile([128, 512], F32, tag="mm")
                nc.tensor.matmul(
                    bc_ps[0:64, :], lhsT=ones32[64:65, :], rhs=denrow[64:65, :],
                    start=True, stop=True,
                )
                recip = pool_r.tile([64, 512], F32, tag="recip")
                nc.vector.reciprocal_approx_fast(recip[:], bc_ps[0:64, :])
                nc.vector.tensor_mul(
                    att_sb[h * 64:(h + 1) * 64, j * 512:(j + 1) * 512],
                    avden[0:64, hc:hc + 512], recip[:],
                )

        # ================= prologue: round-0 projections =================
        qk_proj_interleaved(0)
        for t in range(4):
            v_tile(t)

        # ================= main loop =================
        # PE executes its queue in order, so work that should fill the
        # exp-wait bubbles (O-proj of round j-1, projections of round j+1)
        # is explicitly interleaved between attention blocks, and each AV
        # pair is emitted one block behind its S/exp.
        for j in range(NT512):
            avden = ps_av.tile([128, 1024], F32, tag="avden")
            n_i = 4 * j + 4

            fillers = []
            if j >= 1:
                for t in range(4 * (j - 1), 4 * (j - 1) + 4):
                    fillers.append(("o", t))
            if j + 1 < NT512:
                fillers.append(("qk", j + 1, wq_sb, qT_sb))
                fillers.append(("qk", j + 1, wk_sb, kT_sb))
                for t in range(4 * (j + 1), 4 * (j + 1) + 4):
                    fillers.append(("v", t))
            # distribute fillers over blocks 0..n_i-1 (earliest-ready first)
            per_block = [[] for _ in range(n_i)]
            for k, f in enumerate(fillers):
                per_block[min(n_i - 1, (k * n_i) // max(1, len(fillers)))].append(f)

            def emit_filler(f):
                if f[0] == "o":
                    o_chunk(f[1])
                elif f[0] == "qk":
                    qk_chain(f[1], f[2], f[3])
                else:
                    v_tile(f[1])

            av_pend = None

            def emit_av(args):
                i, off, ncol, first, last, p_sb = args
                nc.tensor.matmul(
                    avden[0:65, off:512],
                    lhsT=v_sb[:, VS * i: VS * i + 65],
                    rhs=p_sb[:, off:512],
                    start=first, stop=last,
                )
                nc.tensor.matmul(
                    avden[0:65, 512 + off:1024],
                    lhsT=v_sb[:, VS * i + 65: VS * i + 130],
                    rhs=p_sb[:, 512:512 + ncol],
                    start=first, stop=last,
                )

            for i in range(n_i):
                m = i - 4 * j          # >= 0 on diagonal blocks
                off = 128 * m if m > 0 else 0
                ncol = 512 - off
                first, last = (i == 0), (i == n_i - 1)
                # both heads causally column-trimmed; one ACT op covers
                # [off, 512+ncol) contiguously with no uninitialized gap.
                s_pair = ps_s.tile([128, 1024], F32, tag="s")
                nc.tensor.matmul(
                    s_pair[:, off:512],
                    lhsT=kT_sb[0:64, i * 128:(i + 1) * 128],
                    rhs=qT_sb[0:64, j * 512 + off:(j + 1) * 512],
                    start=True, stop=True, tile_position=(0, 0),
                )
                nc.tensor.matmul(
                    s_pair[:, 512:512 + ncol],
                    lhsT=kT_sb[64:128, i * 128:(i + 1) * 128],
                    rhs=qT_sb[64:128, j * 512 + off: (j + 1) * 512],
                    start=True, stop=True, tile_position=(64, 0),
                )
                p_sb = pool_p.tile([128, 1024], BF, tag="p")
                nc.scalar.activation(
                    p_sb[:, off:512 + ncol], s_pair[:, off:512 + ncol], Exp, scale=SCALE,
                )
                if m >= 0:  # causal mask on the 128x128 diagonal sub-block
                    nc.vector.tensor_mul(
                        p_sb[:, off:off + 128],
                        p_sb[:, off:off + 128], mask_sb[:],
                    )
                    nc.vector.tensor_mul(
                        p_sb[:, 512:640], p_sb[:, 512:640], mask_sb[:],
                    )
                if av_pend is not None:
                    emit_av(av_pend)
                av_pend = (i, off, ncol, first, last, p_sb)
                for f in per_block[i]:
                    emit_filler(f)
            emit_av(av_pend)

            normalize(j, avden)

        # last round's O-projection has nothing left to hide behind
        for t in range(4 * (NT512 - 1), 4 * NT512):
            o_chunk(t)

        if dbg is not None:
            for name, sb in (("qT", qT_sb), ("kT", kT_sb), ("att", att_sb)):
                nc.sync.dma_start(dbg[name][:], sb[:])


def _build_program(debug_dumps=False):
    nc = bacc.Bacc("TRN2", debug=False, num_devices=N_CORES)
    xT = nc.dram_tensor("xT", [D, T], BF, kind="ExternalInput").ap()
    wq = nc.dram_tensor("wq", [128, D], BF, kind="ExternalInput").ap()
    wk = nc.dram_tensor("wk", [128, D], BF, kind="ExternalInput").ap()
    wv = nc.dram_tensor("wv", [128, D], BF, kind="ExternalInput").ap()
    wo = nc.dram_tensor("wo", [128, D], BF, kind="ExternalInput").ap()
    mask = nc.dram_tensor("mask", [128, 128], BF, kind="ExternalInput").ap()
    y = nc.dram_tensor("y", [T, D], BF, kind="ExternalOutput").ap()
    dbg = None
    if debug_dumps:
        dbg = {
            name: nc.dram_tensor(f"dbg_{name}", [128, T], BF, kind="ExternalOutput").ap()
            for name in ("qT", "kT", "att")
        }

    with tile.TileContext(nc) as tc:
        _kernel(tc, y, xT, wq, wk, wv, wo, mask, dbg=dbg)
    nc.compile()
    return nc


_NC = None


def _get_program():
    global _NC
    if _NC is None:
        _NC = _build_program()
    return _NC


def _rearrange_w(w_cols):
    """[1024, 128] f32 slice of W_qkv -> [128, 1024] bf16 with d-chunk d at
    cols [d*128, (d+1)*128): out[p, d*128 + m] = w_cols[d*128 + p, m]."""
    return np.ascontiguousarray(
        w_cols.reshape(KD, 128, 128).transpose(1, 0, 2).reshape(128, KD * 128)
    ).astype(BF16)


def make_in_maps(x, W_qkv, W_o):
    x2 = np.asarray(x, dtype=np.float32).reshape(T, D)
    W_qkv = np.asarray(W_qkv, dtype=np.float32)
    W_o = np.asarray(W_o, dtype=np.float32)

    xT_bf = np.ascontiguousarray(x2.T).astype(BF16)
    mask = np.triu(np.ones((128, 128), dtype=np.float32)).astype(BF16)

    in_maps = []
    for c in range(N_CORES):
        cs = slice(2 * c * HD, 2 * c * HD + 128)
        in_maps.append({
            "xT": xT_bf,
            "wq": _rearrange_w(W_qkv[:, 0 * D:1 * D][:, cs]),
            "wk": _rearrange_w(W_qkv[:, 1 * D:2 * D][:, cs]),
            "wv": _rearrange_w(W_qkv[:, 2 * D:3 * D][:, cs]),
            "wo": np.ascontiguousarray(W_o[c * 128:(c + 1) * 128, :]).astype(BF16),
            "mask": mask,
        })
    return in_maps


def combine_outputs(results):
    y_full = np.zeros((T, D), dtype=np.float32)
    for c in range(N_CORES):
        y_full += results[c]["y"].astype(np.float32)
    return y_full.reshape(1, T, D)


def kernel(x, W_qkv, W_o):
    from concourse.bass_utils import run_bass_kernel_spmd

    nc = _get_program()
    in_maps = make_in_maps(x, W_qkv, W_o)
    res = run_bass_kernel_spmd(nc, in_maps, core_ids=list(range(N_CORES)))
    return combine_outputs(res.results)

